# revision 24
# baseline (speedup 1.0000x reference)
"""Trainium2 Bass kernel: masked squared-error sum, data-parallel on 8 cores.

    total = sum((target - pred)^2  where target != -1.0)

Full inputs: pred, target f32 (4096, 8192).  Row-sharded: core c takes rows
[c*512, (c+1)*512), viewed as (128 partitions, 32768 free) — a free
contiguous reshape.

The host interleaves target and pred per tile into ONE DRAM tensor
x[P, 2*FREE] so each 128x(2F) tile arrives in a single DMA: TRN2 DVE
instructions only get one semaphore-wait slot (walrus errors with two DMA
waits on a TensorTensor), so both operands must be covered by one DMA sem.

Per tile (t = xt[:, :F], p = xt[:, F:2F]):

    DVE:  diff = t - p                                 (tensor_sub)
    DVE:  md   = (t != -1) * diff                      (scalar_tensor_tensor)
    ACT:  sq   = Square(md), accum_out -> per-partition partial sums

v2 changes vs baseline:
  - input DMAs issued from nc.sync (HWDGE): no Q7 descriptor-gen serial
    cost, and the Sync sequencer clears its preamble ~2.5 us before
    GpSimd does, so the first bytes land earlier.
  - variable tile sizes (small first tiles for an early compute start,
    small last tiles so the post-last-byte TT+STT+ACT chain is short).
  - stats tiles are DMA'd out directly (two tiny HWDGE DMAs) instead of
    being copied into a gather tile first.

Each tile's 128 partial sums land in one column of a (128, NIT/2) stats
tile (two alternating tiles: same-engine WAW at lag 2 is elided by Tile,
lag 1 is not), DMA'd to DRAM per core; the host reduces in float64.
DMA-bound: 32 MiB/core at the ~433 GB/s SBUF-fabric rate => ~77 us/core.
"""

import numpy as np

_C = 8            # cores
_P = 128          # SBUF partitions
_M, _N = 4096, 8192
_FREE = (_M // _C) * _N // _P   # 32768 free elems per partition per core
# Tile free sizes (per operand): small head so DVE starts ~5 us earlier
# (DVE is ~100% busy from its first op to the end, so its start time is the
# kernel's end time), small tail so the last TT+STT+ACT chain is short.
_SIZES = [512, 1024, 2560, 4096, 4096, 4096, 4096, 4096, 4096, 2048, 1024, 512, 512]
assert sum(_SIZES) == _FREE
_NIT = len(_SIZES)
_OFFS = [sum(_SIZES[:i]) for i in range(_NIT)]
_FMAX = max(_SIZES)


def _build():
    import concourse.bass as bass
    import concourse.tile as tile
    from concourse import mybir

    nc = bass.Bass()
    x_d = nc.dram_tensor("x", [_P, 2 * _FREE], mybir.dt.float32, kind="ExternalInput")
    out_d = nc.dram_tensor("out", [_P, _NIT], mybir.dt.float32, kind="ExternalOutput")
    # Measured dead ends: f32->f16 cast-during-DMA drops the SDMA read rate
    # to ~335 GB/s (vs 433 plain) and STT has no 2x uop anyway; HWDGE input
    # DMAs interleave across queues and delay per-tile completion.  So: f32
    # end-to-end, SWDGE, and the mask STT alternates DVE/GpSimd instead.
    f32 = mybir.dt.float32

    # TRN2 compute instructions get ONE semaphore-wait slot (walrus "Too
    # many sync wait commands" otherwise).  Same-engine waits share the
    # engine's own semaphore and merge, so the whole pipeline stays on DVE:
    # each op then carries at most one wait (the DMA RAW for the first
    # consumer, DVE self-waits for the rest).
    with tile.TileContext(nc) as tc:
        ha = (_NIT + 1) // 2   # even tiles -> stats_a
        hb = _NIT // 2         # odd tiles  -> stats_b
        with (
            tc.tile_pool(name="xp", bufs=4) as xp,
            tc.tile_pool(name="mp", bufs=3) as mp,
            tc.tile_pool(name="qp", bufs=2) as qp,
            tc.tile_pool(name="sp", bufs=1) as sp,
        ):
            # Two alternating stats tiles: same-engine WAW at lag 2 is
            # elided by Tile, lag 1 is not — one shared tile would give the
            # ACT a second (self) wait and break the 1-wait limit.
            stats_a = sp.tile([_P, ha], mybir.dt.float32, tag="sa")
            stats_b = sp.tile([_P, hb], mybir.dt.float32, tag="sb")
            for i in range(_NIT):
                F = _SIZES[i]
                o = _OFFS[i]
                xt = xp.tile([_P, 2 * _FMAX], f32, tag="x")
                # SWDGE (gpsimd), NOT HWDGE: all SWDGE DMAs drain FIFO from
                # one logical queue, so tile i's completion sem fires after
                # exactly its own bytes.  HWDGE fans each transfer across
                # several HW queues and the SDMA engines round-robin between
                # queues at packet granularity — the oldest DMA's completion
                # is then delayed by every newer in-flight DMA, which stalls
                # the DVE pipeline (measured: +14 us end-to-end).
                nc.gpsimd.dma_start(
                    xt[:, 0:2 * F], x_d[:, 2 * o:2 * (o + F)]
                )
                t = xt[:, 0:F]
                p = xt[:, F:2 * F]
                md = mp.tile([_P, _FMAX], f32, tag="md")
                sq = qp.tile([_P, 1], mybir.dt.float32, tag="sq")
                # Diff is computed IN PLACE over p's half of the x tile
                # (elementwise same-index in-place is safe on the streaming
                # engines), so there is no separate d pool and no cross-
                # engine WAR on a d slot.  Engine-validity on TRN2 (walrus):
                # STT is DVE-only, TT is DVE-or-Pool, ACTIVATE is ACT-only —
                # and a Pool TT running concurrently with DVE wedged the
                # device (port-mux hazard; Tile's nc.any never routes
                # elementwise to Pool either), so both passes stay on DVE.
                # No memset sync-carrier: Tile emits [DVE self-wait (TT RAW),
                # ACT WAR (md slot)] on the STT; the strip pass drops the
                # self-wait (implied by program order), leaving one wait.
                nc.vector.tensor_sub(p, t, p)
                nc.vector.scalar_tensor_tensor(
                    out=md[:, 0:F], in0=t, scalar=-1.0, in1=p,
                    op0=mybir.AluOpType.not_equal, op1=mybir.AluOpType.mult,
                )
                st = stats_a if i % 2 == 0 else stats_b
                j = i // 2
                nc.scalar.activation(
                    out=sq.broadcast_to((_P, F)), in_=md[:, 0:F],
                    func=mybir.ActivationFunctionType.Square,
                    accum_out=st[:, j:j + 1],
                )
            nc.gpsimd.dma_start(out_d[:, 0:ha], stats_a[:])
            nc.gpsimd.dma_start(out_d[:, ha:_NIT], stats_b[:])

    _strip_implied_dma_waits(nc)
    return nc


def _strip_implied_dma_waits(nc):
    """Tile's add_semaphores is not transitively minimal (see 02-tile.md),
    but walrus on this toolchain allows only ONE sem wait per instruction.
    Build the transitive happens-before closure over semaphore events and
    drop waits that are implied by another wait on the same instruction
    (e.g. a slot-reusing DMA's lane-WAW wait is implied by its DVE WAR wait;
    the tail drain's DVE wait is implied by the out-DMA's lane wait)."""
    fn = nc.m.functions[0]
    cum = {}          # sem name -> cumulative update value so far
    facts = {}        # (sem, cum_value) -> dict sem -> min guaranteed value

    def facts_for_wait(name, value):
        # facts guaranteed once `name` reaches >= value: the recorded event
        # with the smallest cum >= value.
        best = None
        for (s, v), f in facts.items():
            if s == name and v >= value and (best is None or v < best[0]):
                best = (v, f)
        return best[1] if best else {}

    def merge(dst, src):
        for k, v in src.items():
            if dst.get(k, 0) < v:
                dst[k] = v

    for blk in fn.blocks:
        for ins in blk.instructions:
            si = ins.sync_info
            if si is None:
                continue
            fin = {}
            for w in si.on_wait:
                if getattr(w, "wait_mode", "") != "sem-ge-imm":
                    continue
                merge(fin, facts_for_wait(w.ant_name, w.wait_value))
                merge(fin, {w.ant_name: w.wait_value})
            for u in si.on_update:
                prev = cum.get(u.ant_name, 0)
                new = prev + (u.update_value or 0)
                cum[u.ant_name] = new
                f = dict(fin)
                # same-sem monotonicity: inherits the previous value's facts
                merge(f, facts.get((u.ant_name, prev), {}))
                if prev:
                    merge(f, {u.ant_name: prev})
                facts[(u.ant_name, new)] = f

    # Pass 2a: drop same-engine self-waits already satisfied by program
    # order.  Engines are in-order: by the time instruction J on engine E
    # issues, every earlier E-instruction's sem update has fired.  So a wait
    # on sem S with value <= (cumulative updates to S by earlier same-engine
    # instructions) is a no-op and just burns walrus's single wait slot.
    # EXCEPTION: a DMA trigger's sem update is listed on the trigger
    # instruction but fires only when the DMA DATA completes (async) — those
    # updates are NOT implied by program order and must not be counted.
    eng_cum = {}      # (engine, sem) -> cumulative update by that engine
    for blk in fn.blocks:
        for ins in blk.instructions:
            si = ins.sync_info
            if si is None:
                continue
            eng = ins.engine
            is_async_update = type(ins).__name__ in ("InstDMACopy", "InstLoad", "InstSave")
            if si.on_wait and len(si.on_wait) > 1:
                kept = []
                for w in si.on_wait:
                    if (
                        getattr(w, "wait_mode", "") == "sem-ge-imm"
                        and eng_cum.get((eng, w.ant_name), 0) >= w.wait_value
                    ):
                        continue
                    kept.append(w)
                if len(kept) != len(si.on_wait):
                    si.on_wait = kept
                    ins.sync_info = si
            if not is_async_update:
                for u in si.on_update:
                    k = (eng, u.ant_name)
                    eng_cum[k] = eng_cum.get(k, 0) + (u.update_value or 0)

    for blk in fn.blocks:
        for ins in blk.instructions:
            si = ins.sync_info
            if si is None or len(si.on_wait) <= 1:
                continue
            ws = list(si.on_wait)
            if any(getattr(w, "wait_mode", "") != "sem-ge-imm" for w in ws):
                continue
            kept = []
            for i, w in enumerate(ws):
                implied = False
                for j, w2 in enumerate(ws):
                    if i == j:
                        continue
                    f2 = facts_for_wait(w2.ant_name, w2.wait_value)
                    if f2.get(w.ant_name, 0) >= w.wait_value:
                        # mutual implication: keep the lower-indexed one
                        own = facts_for_wait(w.ant_name, w.wait_value)
                        mutual = own.get(w2.ant_name, 0) >= w2.wait_value
                        if not mutual or j < i:
                            implied = True
                            break
                if not implied:
                    kept.append(w)
            if len(kept) != len(ws):
                si.on_wait = kept
                ins.sync_info = si

    # Pass 2b: remove Tile's teardown dma_reset (a ~2 us Pool DGE-drain) and
    # its semaphore RANGE_CLEAR.  Both are redundant here: the walrus-emitted
    # kernel postamble blanket-resets the whole semaphore file (S[3..255])
    # anyway, and the preceding sync.drain already waited for all DMAs.
    last_blk = fn.blocks[-1]
    isa_idx = [
        k for k, ins in enumerate(last_blk.instructions)
        if type(ins).__name__ == "InstISA"
    ]
    if isa_idx:
        k = isa_idx[-1]
        drop = {id(last_blk.instructions[k])}
        for j in range(k - 1, -1, -1):
            ins = last_blk.instructions[j]
            if type(ins).__name__ == "InstDrain" and str(ins.engine).endswith("Pool"):
                si = ins.sync_info
                if not si or (not si.on_wait and not si.on_update):
                    drop.add(id(ins))
                break
        last_blk.instructions = [
            ins for ins in last_blk.instructions if id(ins) not in drop
        ]

    # Pass 3: any instruction STILL carrying >1 waits gets the excess spilled
    # onto injected same-engine NOPs placed immediately before it — walrus
    # allows one wait per instruction, and same-engine program order makes
    # the NOP's wait equivalent to carrying it on the instruction itself.
    import concourse.mybir as mybir
    nop_n = 0
    for blk in fn.blocks:
        lst = list(blk.instructions)
        out = []
        for ins in lst:
            si = ins.sync_info
            if si is not None and len(si.on_wait) > 1:
                ws = list(si.on_wait)
                for w in ws[:-1]:
                    out.append(mybir.InstNoOp(
                        name=f"nop_xwait_{nop_n}",
                        sync_info=mybir.SyncInfo(on_wait=[w], on_update=[]),
                        engine=ins.engine,
                        bass_nofuse=True,
                    ))
                    nop_n += 1
                si.on_wait = ws[-1:]
                ins.sync_info = si
            out.append(ins)
        if len(out) != len(lst):
            blk.instructions = out


def _shard(pred, target):
    pred_r = np.ascontiguousarray(pred, dtype=np.float32).reshape(_C, _P, _FREE)
    targ_r = np.ascontiguousarray(target, dtype=np.float32).reshape(_C, _P, _FREE)
    x = np.empty((_C, _P, 2 * _FREE), dtype=np.float32)
    for i in range(_NIT):
        F, o = _SIZES[i], _OFFS[i]
        x[:, :, 2 * o:2 * o + F] = targ_r[:, :, o:o + F]
        x[:, :, 2 * o + F:2 * (o + F)] = pred_r[:, :, o:o + F]
    return [{"x": x[c]} for c in range(_C)]


def run(pred, target, **spmd_kwargs):
    """Build + run on all 8 cores; returns (scalar_output, BassKernelResults)."""
    from concourse.bass_utils import run_bass_kernel_spmd

    nc = _build()
    res = run_bass_kernel_spmd(
        nc, _shard(pred, target), core_ids=list(range(_C)), **spmd_kwargs
    )
    total = 0.0
    for c in range(_C):
        total += res.results[c]["out"].astype(np.float64).sum()
    return np.array(total, dtype=np.float32), res


def kernel(pred: np.ndarray, target: np.ndarray) -> np.ndarray:
    out, _ = run(pred, target)
    return out
